# revision 1
# baseline (speedup 1.0000x reference)
"""Biaffine kernel for Trainium2, 8-core SPMD.

Math (reference):
    out[b,x,y,o] = bwn0 * sum_{i,j<=512} x1b[b,x,i] W_bil[o,i,j] x2b[b,y,j]
                 + bwn1 * (x1@W_lin[:512] [b,x,o] + x2@W_lin[512:] [b,y,o] + b_lin[o])
    with x1b/x2b = x append-ones, bwn = softmax(bw).

Decomposition used here (exact):
    out[b,x,y,o] = sum_{j<512} x2[b,y,j] * UT[b,o][j,x]      (step B, PE)
                 + D1[b,o,x]                                  (K=1 matmul fold)
                 + D2[b,y,o]                                  (per-partition scalar add)
    UT[b,o][j,x] = sum_{i<512} (bwn0*W_bil[o,i,j]) * x1[b,x,i]          (step A, PE)
    D1[b,o,x]    = sum_i x1[b,x,i]*G[i,o] + g0[o],  G = bwn0*W_bil[o,:,512] + bwn1*W_lin[:512,o]
    D2[b,y,o]    = sum_j x2[b,y,j]*V[j,o],          V = bwn0*W_bil[o,512,:] + bwn1*W_lin[512:,o]
    g0[o]        = bwn0*W_bil[o,512,512] + bwn1*b_lin[o]

Sharding: tensor-parallel over O (128 output channels -> 16 per core).
Matmuls run as float32r (fp32 storage, TF32-like PE datapath, ~1e-4 rel err,
~4x faster than strict fp32).
"""

import numpy as np

import concourse.bass as bass
import concourse.mybir as mybir
import concourse.tile as tile
from concourse.bass_utils import run_bass_kernel_spmd

B, L, D, O = 4, 256, 512, 128
N_CORES = 8
O_LOC = O // N_CORES          # 16 output channels per core
N_O2 = O_LOC // 2             # 8 o-pairs per core
F32 = mybir.dt.float32
F32R = mybir.dt.float32r


# --------------------------------------------------------------------------
# Workaround: this container's walrus build accepts only ONE sync wait per
# instruction ("Too many sync wait commands").  Tile's wait assignment can
# attach several.  Post-pass: hoist extra waits onto InstEventSemaphore
# wait-carriers inserted immediately before the instruction on the same
# engine stream (same stall point, identical semantics).
_WS_CTR = [0]


def _split_multi_waits(nc):
    for f in nc.m.functions:
        for blk in f.blocks:
            insts = blk.instructions
            new = []
            changed = False
            for inst in insts:
                si = inst.sync_info
                waits = list(si.on_wait) if (si and si.on_wait) else []
                if len(waits) > 1:
                    for w in waits[:-1]:
                        _WS_CTR[0] += 1
                        carrier = mybir.InstEventSemaphore(
                            name=f"waitsplit_{_WS_CTR[0]}", ins=[], outs=[]
                        )
                        carrier.engine = inst.engine
                        carrier.sync_info = mybir.SyncInfo(on_wait=[w], on_update=[])
                        new.append(carrier)
                    si.on_wait = [waits[-1]]
                    changed = True
                new.append(inst)
            if changed:
                blk.instructions = new


# --------------------------------------------------------------------------
def build_nc(split_waits=True, n_o2=N_O2):
    nc = bass.Bass("TRN2", target_bir_lowering=False, debug=False,
                   num_devices=N_CORES)

    WM = nc.dram_tensor("WM", [O_LOC, D, D], F32R, kind="ExternalInput").ap()
    X1T = nc.dram_tensor("X1T", [D, B * L], F32R, kind="ExternalInput").ap()
    X2T = nc.dram_tensor("X2T", [D, B * L], F32R, kind="ExternalInput").ap()
    G = nc.dram_tensor("G", [D, O_LOC], F32R, kind="ExternalInput").ap()
    V = nc.dram_tensor("V", [D, O_LOC], F32R, kind="ExternalInput").ap()
    G0 = nc.dram_tensor("G0", [O_LOC, 1], F32, kind="ExternalInput").ap()
    ONES = nc.dram_tensor("ONES", [128, 128], F32R, kind="ExternalInput").ap()
    OUT = nc.dram_tensor("OUT", [B, O_LOC, L, L], F32, kind="ExternalOutput").ap()

    with tile.TileContext(nc) as tc:
        with (
            tc.tile_pool(name="const", bufs=1) as cst,
            tc.tile_pool(name="w", bufs=2) as wpool,
            tc.tile_pool(name="ut", bufs=2) as utpool,
            tc.tile_pool(name="cs", bufs=4) as cspool,
        ):
            # ---- resident inputs -------------------------------------------------
            X1s = cst.tile([128, 4, B * L], F32R, tag="x1s")     # [i%128, it, b*256+x]
            nc.sync.dma_start(out=X1s[:], in_=X1T.rearrange("(it p) c -> p it c", p=128))
            X2s = cst.tile([128, 4, B * L], F32R, tag="x2s")     # [j%128, jt, b*256+y]
            nc.sync.dma_start(out=X2s[:], in_=X2T.rearrange("(jt p) c -> p jt c", p=128))
            Gs = cst.tile([128, 4, O_LOC], F32R, tag="gs")
            nc.sync.dma_start(out=Gs[:], in_=G.rearrange("(it p) o -> p it o", p=128))
            Vs = cst.tile([128, 4, O_LOC], F32R, tag="vs")
            nc.sync.dma_start(out=Vs[:], in_=V.rearrange("(jt p) o -> p jt o", p=128))
            g0s = cst.tile([O_LOC, 1], F32, tag="g0s")
            nc.sync.dma_start(out=g0s[:], in_=G0[:])
            onesAll = cst.tile([128, 128], F32R, tag="ones")
            nc.sync.dma_start(out=onesAll[:], in_=ONES[:])

            # persistent D-term tiles
            D1T = [cst.tile([O_LOC, L], F32R, tag=f"d1t{b}", name=f"d1t{b}") for b in range(B)]
            D2C = [cst.tile([128, 2, O_LOC], F32, tag=f"d2c{b}", name=f"d2c{b}") for b in range(B)]
            # rhs rows for the K=1 fold: partition 32*b holds D1T[b] flattened
            # o-major, so rhs5 = D1F[32b, o2*512:(o2+1)*512] covers an o-pair.
            D1F = cst.tile([128, O_LOC * L], F32R, tag="d1f")

            # ---- precompute D-terms ---------------------------------------------
            with tc.tile_pool(name="psD", bufs=1, space="PSUM") as psD:
                for b in range(B):
                    pd1 = psD.tile([O_LOC, L], F32, tag="pd1")
                    for it in range(4):
                        nc.tensor.matmul(
                            pd1[:],
                            lhsT=Gs[:, it, :],
                            rhs=X1s[:, it, b * L:(b + 1) * L],
                            start=(it == 0), stop=(it == 3),
                        )
                    nc.vector.tensor_scalar_add(D1T[b][:], pd1[:], g0s[:, 0:1])
                    for o in range(O_LOC):
                        nc.sync.dma_start(
                            out=D1F[32 * b:32 * b + 1, o * L:(o + 1) * L],
                            in_=D1T[b][o:o + 1, :],
                        )
                    for yt in range(2):
                        pd2 = psD.tile([128, O_LOC], F32, tag="pd2")
                        for jt in range(4):
                            nc.tensor.matmul(
                                pd2[:],
                                lhsT=X2s[:, jt, b * L + yt * 128: b * L + (yt + 1) * 128],
                                rhs=Vs[:, jt, :],
                                start=(jt == 0), stop=(jt == 3),
                            )
                        nc.vector.tensor_copy(out=D2C[b][:, yt, :], in_=pd2[:])

            # ---- main loop over o-pairs -----------------------------------------
            psA = ctx_psA = tc.tile_pool(name="psA", bufs=4, space="PSUM")
            psB = ctx_psB = tc.tile_pool(name="psB", bufs=4, space="PSUM")
            psA = psA.__enter__()
            psB = psB.__enter__()
            for o2 in range(n_o2):
                Ws = wpool.tile([128, 2, 4, D], F32R, tag="ws")   # [i%128, oi, it, j]
                nc.sync.dma_start(
                    out=Ws[:],
                    in_=WM[2 * o2:2 * o2 + 2].rearrange("oi (it p) j -> p oi it j", p=128),
                )
                # step A: UT[bp][j, (jt, oi, b2*256+x)] for this o-pair
                UT = [utpool.tile([128, 4, 2, 512], F32R, tag=f"utp{bp}", name=f"utp{bp}")
                      for bp in range(2)]
                for oi in range(2):
                    for jt in range(4):
                        for bp in range(2):
                            pa = psA.tile([128, 512], F32, tag="pa")
                            for it in range(4):
                                nc.tensor.matmul(
                                    pa[:],
                                    lhsT=Ws[:, oi, it, jt * 128:(jt + 1) * 128],
                                    rhs=X1s[:, it, bp * 512:(bp + 1) * 512],
                                    start=(it == 0), stop=(it == 3),
                                )
                            nc.vector.tensor_copy(
                                out=UT[bp][:, jt, oi, :], in_=pa[:])
                # step B: out[y, (oi, x)] per (b, yt)
                for b in range(B):
                    for yt in range(2):
                        pb = psB.tile([128, 512], F32, tag="pb")
                        bp, b2 = divmod(b, 2)
                        for jt in range(4):
                            nc.tensor.matmul(
                                pb[:],
                                lhsT=X2s[:, jt, b * L + yt * 128: b * L + (yt + 1) * 128],
                                rhs=UT[bp][:, jt, :, b2 * 256:(b2 + 1) * 256],
                                start=(jt == 0), stop=False,
                            )
                        # + D1 (x-dependent, broadcast over y): K=1 rank-1 fold
                        nc.tensor.matmul(
                            pb[:],
                            lhsT=onesAll[32 * b:32 * b + 1, 0:128],
                            rhs=D1F[32 * b:32 * b + 1, o2 * 512:(o2 + 1) * 512],
                            start=False, stop=True,
                            tile_position=(32 * b, 0),
                        )
                        cs = cspool.tile([128, 512], F32, tag="cs")
                        for oi in range(2):
                            og = 2 * o2 + oi
                            # + D2 (y-dependent per-partition scalar), evict
                            nc.vector.tensor_scalar_add(
                                cs[:, oi * 256:(oi + 1) * 256],
                                pb[:, oi * 256:(oi + 1) * 256],
                                D2C[b][:, yt, og:og + 1],
                            )
                            nc.sync.dma_start(
                                out=OUT[b, og, yt * 128:(yt + 1) * 128, :],
                                in_=cs[:, oi * 256:(oi + 1) * 256],
                            )

            ctx_psB.__exit__(None, None, None)
            ctx_psA.__exit__(None, None, None)

    if split_waits:
        _split_multi_waits(nc)
    return nc


_NC_CACHE = None


def _get_nc():
    global _NC_CACHE
    if _NC_CACHE is None:
        _NC_CACHE = build_nc()
    return _NC_CACHE


def _prep_inputs(x1, x2, bw, W_bil, W_lin, b_lin):
    """Host-side glue: softmax of the 2-vector, per-core slicing/layout."""
    x1 = np.asarray(x1, np.float32)
    x2 = np.asarray(x2, np.float32)
    bw = np.asarray(bw, np.float64)
    W_bil = np.asarray(W_bil, np.float32)
    W_lin = np.asarray(W_lin, np.float32)
    b_lin = np.asarray(b_lin, np.float32)

    e = np.exp(bw - bw.max())
    bwn = (e / e.sum()).astype(np.float32)
    bwn0, bwn1 = float(bwn[0]), float(bwn[1])

    x1T = np.ascontiguousarray(x1.transpose(2, 0, 1).reshape(D, B * L))
    x2T = np.ascontiguousarray(x2.transpose(2, 0, 1).reshape(D, B * L))
    ones = np.ones((128, 128), np.float32)

    in_maps = []
    for c in range(N_CORES):
        o_sl = slice(c * O_LOC, (c + 1) * O_LOC)
        Wb = W_bil[o_sl]                                   # [16, 513, 513]
        WM = np.ascontiguousarray(bwn0 * Wb[:, :D, :D])
        G = np.ascontiguousarray(bwn0 * Wb[:, :D, D].T + bwn1 * W_lin[:D, o_sl])
        V = np.ascontiguousarray(bwn0 * Wb[:, D, :D].T + bwn1 * W_lin[D:, o_sl])
        G0 = (bwn0 * Wb[:, D, D] + bwn1 * b_lin[o_sl]).reshape(O_LOC, 1)
        in_maps.append({
            "WM": WM, "X1T": x1T, "X2T": x2T,
            "G": G.astype(np.float32), "V": V.astype(np.float32),
            "G0": np.ascontiguousarray(G0, dtype=np.float32), "ONES": ones,
        })
    return in_maps


def _assemble(results):
    out = np.empty((B, L, L, O), np.float32)
    for c in range(N_CORES):
        # per-core OUT is [b, o_local, y, x] -> full is [b, x, y, o]
        out[:, :, :, c * O_LOC:(c + 1) * O_LOC] = \
            results[c]["OUT"].transpose(0, 3, 2, 1)
    return out


def kernel(**inputs):
    in_maps = _prep_inputs(**inputs)
    nc = _get_nc()
    res = run_bass_kernel_spmd(nc, in_maps, list(range(N_CORES)))
    return _assemble(res.results)

